# revision 1
# baseline (speedup 1.0000x reference)
"""Bass/Trainium2 kernel for nn_DisentangleLayer (FactorGCN-style GNN layer).

Math (per reference):
  h    = x @ W_lin + b_lin                    [N, 256]
  h_em = x @ emb_w + emb_b                    [N, 64]
  s_src = h @ a_src.T ; s_dst = h @ a_dst.T   [N, 4]    (att_w = [a_src | a_dst])
  e    = sigmoid(s_src[src] + s_dst[dst] + att_b)       [E, 4]
  ev   = exp(e - max(e))     (max subtraction cancels in the normalized
                              ratio below; sigmoid output is bounded so no
                              overflow risk -> we skip it)
  denom = segsum_src(ev)                       [N, 4]
  out[n, 64l:64l+64] = segsum_src(ev_l * h_em[dst]) / denom[n, l]

Strategy:
  * Host shards edges by src-range across 8 cores (each core owns 12500
    nodes' outputs; no cross-core reduction needed).
  * Per core, edges are sorted by src and mapped to dense ranks; ranks are
    grouped into 128-node windows; each window's edge list is padded to a
    fixed number of 128-edge tiles (uniform across cores -> one compiled
    program).
  * Device: phase A computes a packed per-node table
    [h_em(64) | s_dst+att_b(4) | s_src(4) | pad] (f32, 512B rows) for all
    N nodes (replicated per core).  Edge phase gathers table rows by dst
    via indirect DMA, forms per-tile one-hot matrices S (edges x ranks)
    and S^T on-chip, and uses TensorE matmuls for the per-window segment
    sums of [ev*h_em | ev]; normalization by denom happens per window.
"""

import math
import numpy as np
from contextlib import ExitStack

P = 128
CORES = 8
IN_F = 256
D_EM = 64
L = 4

_PATCHED = False


def _apply_tile_patch():
    """walrus in this env rejects >1 sem-wait on one instruction; split the
    TileContext exit-drain waits across single-wait nops."""
    global _PATCHED
    if _PATCHED:
        return
    _PATCHED = True
    import concourse.tile as tile_mod
    import concourse.mybir as mybir
    from concourse.vector_clock import ScopedClock

    def _drain_and_barrier(self, tick_clock, wait_clock):
        nop = self.nc.sync.nop()
        wait_clock.add_sem_waits(nop.ins, ScopedClock({None: tick_clock.global_clock}))
        si = nop.ins.sync_info
        waits = list(si.on_wait) if si is not None else []
        if len(waits) > 1:
            si.on_wait = waits[:1]
            nop.ins.sync_info = si
            for i in range(1, len(waits)):
                extra = self.nc.sync.nop()
                extra.ins.sync_info = mybir.SyncInfo(
                    on_wait=waits[i : i + 1], on_update=[]
                )
        self.nc.sync.drain()
        self.nc.all_engine_barrier()
        assert self.sems is not None
        popped = self.nc._tile_sem_poison_stack.pop()
        assert popped is self._sem_poison
        self.nc.clear_and_free_semaphores(list(self.sems.allocated().values()))
        self.nc.all_engine_barrier()

    tile_mod.TileContext._drain_and_barrier = _drain_and_barrier


# ----------------------------------------------------------------------------
# host-side sharding / stream building
# ----------------------------------------------------------------------------

def _host_prep(src, dst, n_nodes, n_cores):
    """Shard edges by src range, sort by src, build per-core device streams.

    Returns (cfg, per_core) where per_core[c] is a dict of numpy arrays and
    cfg holds the uniform shape parameters.
    """
    NV = n_nodes // n_cores
    NW = (NV + P - 1) // P
    src = np.asarray(src)
    dst = np.asarray(dst)

    cores = []
    for c in range(n_cores):
        lo = c * NV
        sel = (src >= lo) & (src < lo + NV)
        es = src[sel].astype(np.int64) - lo
        ed = dst[sel].astype(np.int64)
        order = np.argsort(es, kind="stable")
        es = es[order]
        ed = ed[order]
        u, counts = np.unique(es, return_counts=True)
        K = len(u)
        ranks = np.repeat(np.arange(K, dtype=np.int64), counts)
        w = ranks // P
        cnt_w = np.bincount(w, minlength=NW)
        cores.append((ed, u, K, ranks, w, cnt_w))

    T_w = 1
    for (_, _, _, _, _, cnt_w) in cores:
        T_w = max(T_w, int(math.ceil(cnt_w.max() / P)))

    per_core = []
    for c in range(n_cores):
        ed, u, K, ranks, w, cnt_w = cores[c]
        lo = c * NV
        nslot = T_w * P
        slot_rank = np.full((NW, nslot), -1.0, np.float32)
        slot_dst = np.zeros((NW, nslot), np.int32)
        offs = np.concatenate([[0], np.cumsum(cnt_w)])
        pos = np.arange(len(ed)) - offs[w]
        slot_rank[w, pos] = (ranks % P).astype(np.float32)
        slot_dst[w, pos] = ed.astype(np.int32)

        # device layouts: [128, NW*T_w] with slot (w, i, p) -> [p, w*T_w + i]
        rank_col = (
            slot_rank.reshape(NW, T_w, P).transpose(2, 0, 1).reshape(P, NW * T_w)
        )
        didx = slot_dst.reshape(NW, T_w, P).transpose(2, 0, 1).reshape(P, NW * T_w)

        # ST one-hot bytes: stb[n, (w, i, e)] == 1 iff rank of slot
        # (w, i, e) == n.   (e is the partition index of the edge.)
        stb = np.zeros((P, NW, T_w, P), np.uint8)
        sr = slot_rank.reshape(NW, T_w, P)
        wv, iv, evi = np.nonzero(sr >= 0)
        nv = sr[wv, iv, evi].astype(np.int64)
        stb[nv, wv, iv, evi] = 1
        stb = stb.reshape(P, NW * T_w * P)

        u_pad = np.zeros(NW * P, np.int32)
        u_pad[:K] = (u + lo).astype(np.int32)
        uidx = u_pad.reshape(NW, P).T.copy()  # [128, NW]

        per_core.append(
            dict(didx=didx, rankc=rank_col, stb=stb, uidx=uidx, u=u, K=K)
        )

    cfg = dict(NV=NV, NW=NW, T_w=T_w)
    return cfg, per_core


# ----------------------------------------------------------------------------
# device program
# ----------------------------------------------------------------------------

def _split_multi_waits(nc):
    """This env's walrus accepts at most ONE sync-wait command per
    instruction.  Move extra waits onto single-wait nops inserted just
    before the instruction on the same engine (same sequencer => identical
    semantics)."""
    import concourse.mybir as mybir

    cnt = 0
    for f in nc.m.functions:
        for blk in f.blocks:
            insts = blk.instructions
            out = []
            changed = False
            for ins in insts:
                si = ins.sync_info
                waits = list(si.on_wait) if si is not None else []
                if len(waits) > 1:
                    changed = True
                    for w in waits[:-1]:
                        cnt += 1
                        nop = mybir.InstNoOp(
                            name=f"wsplit_{cnt}", ins=[], outs=[]
                        )
                        nop.engine = ins.engine
                        nop.sync_info = mybir.SyncInfo(on_wait=[w], on_update=[])
                        out.append(nop)
                    si.on_wait = waits[-1:]
                    ins.sync_info = si
                out.append(ins)
            if changed:
                blk.instructions = out
    return cnt


def _build_nc(N, NW, T_w, TC=16, split_waits=True):
    _apply_tile_patch()
    import concourse.bass as bass
    import concourse.mybir as mybir
    import concourse.tile as tile
    from concourse.masks import make_identity

    f32 = mybir.dt.float32
    i32 = mybir.dt.int32
    u8 = mybir.dt.uint8
    AF = mybir.ActivationFunctionType
    OP = mybir.AluOpType
    IOOA = bass.IndirectOffsetOnAxis

    nc = bass.Bass()
    x_d = nc.declare_dram_parameter("x", [N, IN_F], f32, isOutput=False)
    wl_d = nc.declare_dram_parameter("wl", [IN_F, IN_F], f32, isOutput=False)
    aw_d = nc.declare_dram_parameter("aw", [L, 2 * IN_F], f32, isOutput=False)
    ew_d = nc.declare_dram_parameter("ew", [IN_F, D_EM], f32, isOutput=False)
    bl_d = nc.declare_dram_parameter("bl", [IN_F, 1], f32, isOutput=False)
    embb_d = nc.declare_dram_parameter("embb", [1, D_EM], f32, isOutput=False)
    attb_d = nc.declare_dram_parameter("attb", [1, L], f32, isOutput=False)
    didx_d = nc.declare_dram_parameter("didx", [P, NW * T_w], i32, isOutput=False)
    rankc_d = nc.declare_dram_parameter("rankc", [P, NW * T_w], f32, isOutput=False)
    stb_d = nc.declare_dram_parameter("stb", [P, NW * T_w * P], u8, isOutput=False)
    uidx_d = nc.declare_dram_parameter("uidx", [P, NW], i32, isOutput=False)
    iota_d = nc.declare_dram_parameter("iota_mat", [P, P], f32, isOutput=False)
    descale_d = nc.declare_dram_parameter("descale", [P, 1], f32, isOutput=False)
    out_d = nc.declare_dram_parameter("out", [NW * P, 4 * D_EM], f32, isOutput=True)

    tbl = nc.dram_tensor("tbl", [N, P], f32)  # [h_em(64)|s_dst+attb(4)|s_src(4)|0]

    ntileA = (N + P - 1) // P
    chunks = []
    k0 = 0
    while k0 < T_w:
        chunks.append((k0, min(TC, T_w - k0)))
        k0 += TC

    with ExitStack() as ctx:
        tc = ctx.enter_context(tile.TileContext(nc))
        const = ctx.enter_context(tc.tile_pool(name="const", bufs=1))

        ident = const.tile([P, P], f32)
        make_identity(nc, ident[:])
        iota = const.tile([P, P], f32)
        nc.sync.dma_start(out=iota[:], in_=iota_d[:])
        descale = const.tile([P, 1], f32)
        nc.sync.dma_start(out=descale[:], in_=descale_d[:])

        # ---- fold weights: Wp[ic] = [emb_w | W@a_dst.T | W@a_src.T | 0] ----
        WT = [[const.tile([P, P], f32, name=f"WT_{j}_{i}") for i in range(2)] for j in range(2)]
        adT = [const.tile([P, L], f32, name=f"adT_{j}") for j in range(2)]
        asT = [const.tile([P, L], f32, name=f"asT_{j}") for j in range(2)]
        blT = [const.tile([P, 1], f32, name=f"blT_{j}") for j in range(2)]
        Wp = [const.tile([P, P], f32, name=f"Wp_{i}") for i in range(2)]
        bias_row = const.tile([1, P], f32)
        ones1 = const.tile([1, P], f32)
        bias_bc = const.tile([P, P], f32)
        attb_sb = const.tile([1, L], f32)

        with (
            tc.tile_pool(name="setup_sb", bufs=2) as ssb,
            tc.tile_pool(name="setup_ps", bufs=2, space="PSUM") as sps,
        ):
            for jc in range(2):
                nc.sync.dma_start(
                    out=adT[jc][:],
                    in_=aw_d[:, IN_F + jc * P : IN_F + (jc + 1) * P].transpose([1, 0]),
                )
                nc.sync.dma_start(
                    out=asT[jc][:],
                    in_=aw_d[:, jc * P : (jc + 1) * P].transpose([1, 0]),
                )
                nc.sync.dma_start(out=blT[jc][:], in_=bl_d[jc * P : (jc + 1) * P, :])
                for ic in range(2):
                    wt = ssb.tile([P, P], f32)
                    nc.sync.dma_start(
                        out=wt[:],
                        in_=wl_d[ic * P : (ic + 1) * P, jc * P : (jc + 1) * P],
                    )
                    tp = sps.tile([P, P], f32, space="PSUM")
                    nc.tensor.transpose(out=tp[:], in_=wt[:], identity=ident[:])
                    nc.vector.tensor_copy(out=WT[jc][ic][:], in_=tp[:])

            for ic in range(2):
                nc.gpsimd.memset(Wp[ic][:], 0)
                nc.sync.dma_start(
                    out=Wp[ic][:, 0:D_EM], in_=ew_d[ic * P : (ic + 1) * P, :]
                )
                wd_ps = sps.tile([P, 2 * L], f32, space="PSUM")
                for t, rhs_t in ((0, adT), (1, asT)):
                    for jc in range(2):
                        nc.tensor.matmul(
                            out=wd_ps[:, t * L : (t + 1) * L],
                            lhsT=WT[jc][ic][:],
                            rhs=rhs_t[jc][:],
                            start=(jc == 0),
                            stop=(jc == 1),
                        )
                nc.vector.tensor_copy(
                    out=Wp[ic][:, D_EM : D_EM + 2 * L], in_=wd_ps[:]
                )

            bias_ps = sps.tile([1, 2 * L], f32, space="PSUM")
            for t, rhs_t in ((0, adT), (1, asT)):
                for jc in range(2):
                    nc.tensor.matmul(
                        out=bias_ps[:, t * L : (t + 1) * L],
                        lhsT=blT[jc][:],
                        rhs=rhs_t[jc][:],
                        start=(jc == 0),
                        stop=(jc == 1),
                    )
            nc.gpsimd.memset(bias_row[:], 0)
            nc.sync.dma_start(out=bias_row[:, 0:D_EM], in_=embb_d[:])
            nc.sync.dma_start(out=attb_sb[:], in_=attb_d[:])
            nc.vector.tensor_tensor(
                out=bias_row[:, D_EM : D_EM + L],
                in0=bias_ps[:, 0:L],
                in1=attb_sb[:],
                op=OP.add,
            )
            nc.vector.tensor_copy(
                out=bias_row[:, D_EM + L : D_EM + 2 * L], in_=bias_ps[:, L : 2 * L]
            )
            # broadcast bias_row across partitions via K=1 matmul
            nc.gpsimd.memset(ones1[:], 1.0)
            bb_ps = sps.tile([P, P], f32, space="PSUM")
            nc.tensor.matmul(
                out=bb_ps[:], lhsT=ones1[:], rhs=bias_row[:], start=True, stop=True
            )
            nc.vector.tensor_copy(out=bias_bc[:], in_=bb_ps[:])

        # ---- phase A: build tbl[N, 128] ----
        with (
            tc.tile_pool(name="xa", bufs=3) as xa,
            tc.tile_pool(name="xt", bufs=3) as xtp,
            tc.tile_pool(name="stg", bufs=3) as stg,
            tc.tile_pool(name="psT", bufs=2, space="PSUM") as psT,
            tc.tile_pool(name="psM", bufs=2, space="PSUM") as psM,
        ):
            for i in range(ntileA):
                r0 = i * P
                pp = min(P, N - r0)
                xt = xa.tile([P, IN_F], f32)
                nc.sync.dma_start(out=xt[:pp, :], in_=x_d[r0 : r0 + pp, :])
                xTs = []
                for jc in range(2):
                    tp = psT.tile([P, P], f32, space="PSUM")
                    nc.tensor.transpose(
                        out=tp[:, :pp],
                        in_=xt[:pp, jc * P : (jc + 1) * P],
                        identity=ident[:pp, :pp],
                    )
                    xT = xtp.tile([P, P], f32)
                    nc.scalar.copy(out=xT[:, :pp], in_=tp[:, :pp])
                    xTs.append(xT)
                tab_ps = psM.tile([P, P], f32, space="PSUM")
                for jc in range(2):
                    nc.tensor.matmul(
                        out=tab_ps[:pp, :],
                        lhsT=xTs[jc][:, :pp],
                        rhs=Wp[jc][:],
                        start=(jc == 0),
                        stop=(jc == 1),
                    )
                st = stg.tile([P, P], f32)
                nc.vector.tensor_tensor(
                    out=st[:pp, :], in0=tab_ps[:pp, :], in1=bias_bc[:pp, :], op=OP.add
                )
                nc.sync.dma_start(out=tbl[r0 : r0 + pp, :], in_=st[:pp, :])

        # ---- upfront: s_src gather + streams ----
        uix = const.tile([P, NW], i32)
        nc.sync.dma_start(out=uix[:], in_=uidx_d[:])
        ssrc = const.tile([P, NW, L], f32)
        for w in range(NW):
            nc.gpsimd.indirect_dma_start(
                out=ssrc[:, w, :],
                out_offset=None,
                in_=tbl[:, :],
                in_offset=IOOA(ap=uix[:, w : w + 1], axis=0),
                element_offset=D_EM + L,
            )
        didx_sb = const.tile([P, NW * T_w], i32)
        nc.sync.dma_start(out=didx_sb[:], in_=didx_d[:])
        rankc_sb = const.tile([P, NW * T_w], f32)
        nc.sync.dma_start(out=rankc_sb[:], in_=rankc_d[:])

        # ---- edge phase ----
        with (
            tc.tile_pool(name="g", bufs=3) as gpool,
            tc.tile_pool(name="stb", bufs=3) as stbp,
            tc.tile_pool(name="st", bufs=3) as stp,
            tc.tile_pool(name="s", bufs=3) as sp,
            tc.tile_pool(name="z", bufs=4) as zp,
            tc.tile_pool(name="rev", bufs=2) as revp,
            tc.tile_pool(name="onorm", bufs=2) as onp,
            tc.tile_pool(name="psZ", bufs=3, space="PSUM") as psZ,
            tc.tile_pool(name="psU", bufs=2, space="PSUM") as psU,
        ):
            for w in range(NW):
                U_ps = psU.tile([P, 4 * D_EM + L], f32, space="PSUM")
                for (k0, tcw) in chunks:
                    c0 = w * T_w + k0
                    G = gpool.tile([P, TC, P], f32)
                    for i in range(tcw):
                        nc.gpsimd.indirect_dma_start(
                            out=G[:, i, :],
                            out_offset=None,
                            in_=tbl[:, :],
                            in_offset=IOOA(
                                ap=didx_sb[:, c0 + i : c0 + i + 1], axis=0
                            ),
                        )
                    stbits = stbp.tile([P, TC, P], u8)
                    nc.sync.dma_start(
                        out=stbits[:, :tcw, :],
                        in_=stb_d[:, c0 * P : (c0 + tcw) * P],
                    )
                    ST = stp.tile([P, TC, P], f32)
                    nc.scalar.copy(out=ST[:, :tcw, :], in_=stbits[:, :tcw, :])
                    S = sp.tile([P, TC, P], f32)
                    nc.vector.tensor_tensor(
                        out=S[:, :tcw, :],
                        in0=rankc_sb[:, c0 : c0 + tcw]
                        .unsqueeze(2)
                        .to_broadcast([P, tcw, P]),
                        in1=iota[:].unsqueeze(1).to_broadcast([P, tcw, P]),
                        op=OP.is_equal,
                    )
                    se_ps = psZ.tile([P, TC, L], f32, space="PSUM")
                    for i in range(tcw):
                        nc.tensor.matmul(
                            out=se_ps[:, i, :],
                            lhsT=ST[:, i, :],
                            rhs=ssrc[:, w, :],
                            start=True,
                            stop=True,
                        )
                    zt = zp.tile([P, TC, L], f32)
                    nc.vector.tensor_tensor(
                        out=zt[:, :tcw, :],
                        in0=se_ps[:, :tcw, :],
                        in1=G[:, :tcw, D_EM : D_EM + L],
                        op=OP.add,
                    )
                    sg = zp.tile([P, TC, L], f32)
                    nc.scalar.activation(
                        out=sg[:, :tcw, :], in_=zt[:, :tcw, :], func=AF.Sigmoid
                    )
                    Rev = revp.tile([P, TC, 4 * D_EM + L], f32)
                    nc.scalar.activation(
                        out=Rev[:, :tcw, 4 * D_EM : 4 * D_EM + L],
                        in_=sg[:, :tcw, :],
                        func=AF.Exp,
                    )
                    nc.vector.tensor_tensor(
                        out=Rev[:, :tcw, 0 : 4 * D_EM].rearrange(
                            "p t (l d) -> p t l d", l=L
                        ),
                        in0=G[:, :tcw, 0:D_EM]
                        .unsqueeze(2)
                        .to_broadcast([P, tcw, L, D_EM]),
                        in1=Rev[:, :tcw, 4 * D_EM : 4 * D_EM + L]
                        .unsqueeze(3)
                        .to_broadcast([P, tcw, L, D_EM]),
                        op=OP.mult,
                    )
                    for i in range(tcw):
                        nc.tensor.matmul(
                            out=U_ps[:, :],
                            lhsT=S[:, i, :],
                            rhs=Rev[:, i, :],
                            start=(k0 == 0 and i == 0),
                            stop=(k0 + tcw == T_w and i == tcw - 1),
                        )
                dn = onp.tile([P, L], f32)
                nc.vector.tensor_scalar(
                    out=dn[:],
                    in0=U_ps[:, 4 * D_EM : 4 * D_EM + L],
                    scalar1=1e-30,
                    scalar2=None,
                    op0=OP.add,
                )
                dnr = onp.tile([P, L], f32)
                nc.vector.reciprocal(out=dnr[:], in_=dn[:])
                ot = onp.tile([P, 4 * D_EM], f32)
                nc.vector.tensor_tensor(
                    out=ot[:].rearrange("p (l d) -> p l d", l=L),
                    in0=U_ps[:, 0 : 4 * D_EM].rearrange("p (l d) -> p l d", l=L),
                    in1=dnr[:].unsqueeze(2).to_broadcast([P, L, D_EM]),
                    op=OP.mult,
                )
                nc.sync.dma_start(out=out_d[w * P : (w + 1) * P, :], in_=ot[:])

    if split_waits:
        _split_multi_waits(nc)
    return nc


# ----------------------------------------------------------------------------
# public entry point
# ----------------------------------------------------------------------------

_NC_CACHE = {}


def _get_nc(N, NW, T_w, TC=16):
    key = (N, NW, T_w, TC)
    if key not in _NC_CACHE:
        _NC_CACHE[key] = _build_nc(N, NW, T_w, TC)
    return _NC_CACHE[key]


def _make_in_maps(x, W_lin, b_lin, att_w, att_b, emb_w, emb_b, per_core, n_cores):
    x = np.ascontiguousarray(np.asarray(x, np.float32))
    shared = dict(
        x=x,
        wl=np.ascontiguousarray(np.asarray(W_lin, np.float32)),
        aw=np.ascontiguousarray(np.asarray(att_w, np.float32)),
        ew=np.ascontiguousarray(np.asarray(emb_w, np.float32)),
        bl=np.ascontiguousarray(np.asarray(b_lin, np.float32).reshape(-1, 1)),
        embb=np.ascontiguousarray(np.asarray(emb_b, np.float32).reshape(1, -1)),
        attb=np.ascontiguousarray(np.asarray(att_b, np.float32).reshape(1, -1)),
        iota_mat=np.broadcast_to(
            np.arange(P, dtype=np.float32), (P, P)
        ).copy(),
        descale=(1.0 / (1 << (np.arange(P) // 16))).astype(np.float32).reshape(P, 1),
    )
    in_maps = []
    for c in range(n_cores):
        m = dict(shared)
        m["didx"] = per_core[c]["didx"]
        m["rankc"] = per_core[c]["rankc"]
        m["stb"] = per_core[c]["stb"]
        m["uidx"] = per_core[c]["uidx"]
        in_maps.append(m)
    return in_maps


def kernel(x, src, dst, W_lin, b_lin, att_w, att_b, emb_w, emb_b):
    from concourse.bass_utils import run_bass_kernel_spmd

    x = np.asarray(x)
    N = x.shape[0]
    cfg, per_core = _host_prep(src, dst, N, CORES)
    nc = _get_nc(N, cfg["NW"], cfg["T_w"])
    in_maps = _make_in_maps(
        x, W_lin, b_lin, att_w, att_b, emb_w, emb_b, per_core, CORES
    )
    res = run_bass_kernel_spmd(nc, in_maps, list(range(CORES)))
    out = np.zeros((N, 4 * D_EM), np.float32)
    NV = cfg["NV"]
    for c in range(CORES):
        K = per_core[c]["K"]
        u = per_core[c]["u"]
        out[c * NV + u] = res.results[c]["out"][:K]
    return out



# revision 17
# speedup vs baseline: 2.1510x; 2.1510x over previous
"""Bass/Trainium2 kernel for nn_DisentangleLayer (FactorGCN-style GNN layer).

Math (per reference):
  h    = x @ W_lin + b_lin                    [N, 256]
  h_em = x @ emb_w + emb_b                    [N, 64]
  s_src = h @ a_src.T ; s_dst = h @ a_dst.T   [N, 4]    (att_w = [a_src | a_dst])
  e    = sigmoid(s_src[src] + s_dst[dst] + att_b)       [E, 4]
  ev   = exp(e)              (the reference's max subtraction cancels in the
                              normalized ratio; sigmoid output is bounded)
  denom = segsum_src(ev)                       [N, 4]
  out[n, 64l:64l+64] = segsum_src(ev_l * h_em[dst]) / denom[n, l]

Weight folding (host, fp32).  Score columns are NEGATED so the device only
ever needs Exp (one activation table, no sigmoid<->exp table reloads):
  Wp  = [emb_w | -W_lin@a_dst.T | -W_lin@a_src.T]       [256, 72]
  b72 = [emb_b | -(b@a_dst.T+att_b) | -b@a_src.T]       [1, 72]
  tbl[n] = x[n] @ Wp + b72 = [hem(64) | -sd(4) | -ssrc(4)]
  u = exp(-(se+sd+b)) ; sigmoid = 1/(1+u) ; ev = exp(sigmoid)

Device strategy (per core; SPMD x8, core owns src nodes [c*NV,(c+1)*NV)):
  * Phase A: tbl [N, 128] bf16 rows (cols 72:128 unwritten pad for the
    256B-aligned gather), stored as 4 dst-chunk DRAM tensors so edge-phase
    gathers of chunk c can start as soon as chunk c is written.
    A separate compact pass in rank order (host passes x[u].T) produces
    per-window ssrc [128, 4] tiles directly into SBUF -- no gather.
  * Edge phase: edges sorted by src are mapped to dense ranks, grouped in
    128-rank windows; within a window slots are grouped by dst-chunk
    (25088 rows each -> int16 indices for dma_gather) and padded to T_c
    128-slot tiles per chunk.  Per window-pair, 4 batched dma_gather
    launches fetch tbl[dst] rows.  One-hot S (slot->rank) and its
    transpose ST stream from DRAM as fp8e4 and feed TensorE directly:
      se = ST.T @ ssrc ; ev = exp(sigmoid(se + sd)) ;
      U  = sum_t S_t.T @ [ev_l*hem | ev] ; out = U[:, :256] / denom.
    The ev_l*hem product: for 2 of 3 windows Act materializes an expanded
    ev (stride-1 last dim -> DVE 2x mode on the multiply); the rest run
    the broadcast multiply directly on DVE, balancing Act vs DVE.
"""

import math
import os
import numpy as np
import ml_dtypes
from contextlib import ExitStack

KDBG = int(os.environ.get("KDBG", "0"))  # 0=full, 1=phaseA only, 2=+gather,
                                         # 3=+se/exp, 4=+revmult, 5=+U/out

P = 128
CORES = 8
IN_F = 256
D_EM = 64
L = 4
NCHUNK = 4

_PATCHED = False


def _apply_tile_patch():
    """walrus in this env rejects >1 sem-wait on one instruction; split the
    TileContext exit-drain waits across single-wait nops."""
    global _PATCHED
    if _PATCHED:
        return
    _PATCHED = True
    import concourse.tile as tile_mod
    import concourse.mybir as mybir
    from concourse.vector_clock import ScopedClock

    def _drain_and_barrier(self, tick_clock, wait_clock):
        nop = self.nc.sync.nop()
        wait_clock.add_sem_waits(nop.ins, ScopedClock({None: tick_clock.global_clock}))
        si = nop.ins.sync_info
        waits = list(si.on_wait) if si is not None else []
        if len(waits) > 1:
            si.on_wait = waits[:1]
            nop.ins.sync_info = si
            for i in range(1, len(waits)):
                extra = self.nc.sync.nop()
                extra.ins.sync_info = mybir.SyncInfo(
                    on_wait=waits[i : i + 1], on_update=[]
                )
        self.nc.sync.drain()
        self.nc.all_engine_barrier()
        assert self.sems is not None
        popped = self.nc._tile_sem_poison_stack.pop()
        assert popped is self._sem_poison
        self.nc.clear_and_free_semaphores(list(self.sems.allocated().values()))
        self.nc.all_engine_barrier()

    tile_mod.TileContext._drain_and_barrier = _drain_and_barrier


def _split_multi_waits(nc):
    """This env's walrus accepts at most ONE sync-wait command per
    instruction.  Move extra waits onto single-wait nops inserted just
    before the instruction on the same engine (same sequencer => identical
    semantics)."""
    import concourse.mybir as mybir

    cnt = 0
    for f in nc.m.functions:
        for blk in f.blocks:
            insts = blk.instructions
            out = []
            changed = False
            for ins in insts:
                si = ins.sync_info
                waits = list(si.on_wait) if si is not None else []
                if len(waits) > 1:
                    changed = True
                    for w in waits[:-1]:
                        cnt += 1
                        nop = mybir.InstNoOp(
                            name=f"wsplit_{cnt}", ins=[], outs=[]
                        )
                        nop.engine = ins.engine
                        nop.sync_info = mybir.SyncInfo(on_wait=[w], on_update=[])
                        out.append(nop)
                    si.on_wait = waits[-1:]
                    ins.sync_info = si
                out.append(ins)
            if changed:
                blk.instructions = out
    return cnt


# ----------------------------------------------------------------------------
# host-side sharding / stream building
# ----------------------------------------------------------------------------

def _wrap_idx16(vals):
    """[n] int array -> dma_gather idx layout [128, n//16] int16:
    value j at [j%16, j//16], replicated across the 8 16-partition groups."""
    n = len(vals)
    m = np.zeros((16, n // 16), np.int16)
    m[np.arange(n) % 16, np.arange(n) // 16] = vals.astype(np.int16)
    return np.tile(m, (8, 1))


def _host_prep(src, dst, n_nodes, n_cores):
    NV = n_nodes // n_cores                       # 12500
    NW = (NV + P - 1) // P                        # 98
    src = np.asarray(src)
    dst = np.asarray(dst)
    chrows = n_nodes // NCHUNK                    # 25000

    cores = []
    for c in range(n_cores):
        lo = c * NV
        sel = (src >= lo) & (src < lo + NV)
        es = src[sel].astype(np.int64) - lo
        ed = dst[sel].astype(np.int64)
        order = np.argsort(es, kind="stable")
        es = es[order]
        ed = ed[order]
        u, counts = np.unique(es, return_counts=True)
        K = len(u)
        ranks = np.repeat(np.arange(K, dtype=np.int64), counts)
        w = ranks // P
        rw = ranks % P
        ch = ed // chrows
        seg = w * NCHUNK + ch
        o2 = np.lexsort((ranks, seg))  # group by (window, chunk)
        ed, w, rw, ch, seg = ed[o2], w[o2], rw[o2], ch[o2], seg[o2]
        cnt_seg = np.bincount(seg, minlength=NW * NCHUNK)
        cores.append((ed, u, K, w, rw, ch, seg, cnt_seg))

    T_c = 1
    for (_, _, _, _, _, _, _, cnt_seg) in cores:
        T_c = max(T_c, int(math.ceil(cnt_seg.max() / P)))
    T_w = NCHUNK * T_c
    NWP = NW // 2
    SEG = 2 * T_c * P  # indices per (pair, chunk) launch

    per_core = []
    for c in range(n_cores):
        ed, u, K, w, rw, ch, seg, cnt_seg = cores[c]
        offs_base = np.concatenate([[0], np.cumsum(cnt_seg)])
        pos = np.arange(len(ed)) - offs_base[seg]   # position within segment
        tc = pos // P                                # tile within chunk
        pp = pos % P                                 # slot partition

        # gather index stream, padded with 0 (gathers a real row; its S
        # column is zero so it contributes nothing)
        idxf = np.zeros((NW, NCHUNK, T_c * P), np.int16)
        idxf[w, ch, pos] = (ed - ch * chrows).astype(np.int16)
        idx16 = np.zeros((P, NWP * NCHUNK * (SEG // 16)), np.int16)
        s16 = SEG // 16
        for q in range(NWP):
            for cc in range(NCHUNK):
                flat = np.concatenate([idxf[2 * q, cc], idxf[2 * q + 1, cc]])
                col0 = (q * NCHUNK + cc) * s16
                idx16[:, col0 : col0 + s16] = _wrap_idx16(flat)

        # one-hot S and ST per window, fp8e4, layout [P, NW, 2, T_w, P]
        stb = np.zeros((P, NW, 2, T_w, P), ml_dtypes.float8_e4m3)
        tg = ch * T_c + tc
        stb[pp, w, 0, tg, rw] = 1.0
        stb[rw, w, 1, tg, pp] = 1.0
        stb = stb.reshape(P, NW * 2 * T_w * P)

        per_core.append(dict(idx16=idx16, stb=stb, u=u, K=K))

    cfg = dict(NV=NV, NW=NW, T_w=T_w)
    return cfg, per_core


# ----------------------------------------------------------------------------
# device program
# ----------------------------------------------------------------------------

def _build_nc(N, NW, T_w):
    _apply_tile_patch()
    import concourse.bass as bass
    import concourse.mybir as mybir
    import concourse.tile as tile
    from concourse.library_config import all_libraries, standard

    f32 = mybir.dt.float32
    bf16 = mybir.dt.bfloat16
    fp8 = mybir.dt.float8e4
    i16 = mybir.dt.int16
    AF = mybir.ActivationFunctionType
    OP = mybir.AluOpType

    T_c = T_w // NCHUNK
    NWP = NW // 2
    SEG = 2 * T_c * P
    S16 = SEG // 16
    CHP = ((N // NCHUNK) + P - 1) // P * P   # 25088 padded chunk rows
    NP = NCHUNK * CHP
    CHT = CHP // P                           # tiles per chunk (196)
    GRP = next(g for g in (6, 5, 4, 7, 3, 2, 1) if CHT % g == 0)
    NGR = CHT // GRP                         # groups per chunk (28)
    assert NW % 2 == 0

    nc = bass.Bass(num_swdge_queues=4)
    xt_d = nc.declare_dram_parameter("xt", [IN_F, NP], bf16, isOutput=False)
    xtu_d = nc.declare_dram_parameter("xtu", [IN_F, NW * P], bf16, isOutput=False)
    wp_d = nc.declare_dram_parameter("wp", [IN_F, 72], bf16, isOutput=False)
    b72_d = nc.declare_dram_parameter("b72", [1, 72], bf16, isOutput=False)
    idx_d = nc.declare_dram_parameter(
        "idx16", [P, NWP * NCHUNK * S16], i16, isOutput=False
    )
    stb_d = nc.declare_dram_parameter(
        "stb", [P, NW * 2 * T_w * P], fp8, isOutput=False
    )
    out_d = nc.declare_dram_parameter("out", [NW * P, 4 * D_EM], f32, isOutput=True)

    tbl = [nc.dram_tensor(f"tbl{c}", [CHP, P], bf16) for c in range(NCHUNK)]

    with ExitStack() as ctx:
        tc = ctx.enter_context(tile.TileContext(nc))
        const = ctx.enter_context(tc.tile_pool(name="const", bufs=1))

        Wp = [const.tile([P, 72], bf16, name=f"Wp{i}") for i in range(2)]
        for i in range(2):
            nc.sync.dma_start(out=Wp[i][:], in_=wp_d[i * P : (i + 1) * P, :])
        b72 = const.tile([1, 72], bf16)
        nc.sync.dma_start(out=b72[:], in_=b72_d[:])
        ones1 = const.tile([1, P], bf16)
        nc.gpsimd.memset(ones1[:], 1.0)
        ssrc = const.tile([P, NW, L], bf16)

        # ---- phase A-u: per-window (negated) ssrc tiles, rank-ordered ----
        with (
            tc.tile_pool(name="xu0", bufs=2) as xu0,
            tc.tile_pool(name="xu1", bufs=2) as xu1,
            tc.tile_pool(name="psu", bufs=2, space="PSUM") as psu,
        ):
            UB = 4  # windows per load group
            for g in range((NW + UB - 1) // UB):
                w0 = g * UB
                nw = min(UB, NW - w0)
                xus = []
                for k, pool_k in ((0, xu0), (1, xu1)):
                    t = pool_k.tile([P, UB * P], bf16)
                    nc.sync.dma_start(
                        out=t[:, : nw * P],
                        in_=xtu_d[k * P : (k + 1) * P, w0 * P : (w0 + nw) * P],
                    )
                    xus.append(t)
                for wi in range(nw):
                    w = w0 + wi
                    ps = psu.tile([P, 72], f32, space="PSUM")
                    for k in range(2):
                        nc.tensor.matmul(
                            out=ps[:],
                            lhsT=xus[k][:, wi * P : (wi + 1) * P],
                            rhs=Wp[k][:],
                            start=(k == 0),
                            stop=False,
                        )
                    nc.tensor.matmul(
                        out=ps[:], lhsT=ones1[:], rhs=b72[:], start=False, stop=True
                    )
                    nc.vector.tensor_copy(
                        out=ssrc[:, w, :], in_=ps[:, D_EM + L : D_EM + 2 * L]
                    )

        # ---- phase A: tbl rows (chunk-major so gathers can pipeline) ----
        with (
            tc.tile_pool(name="xa0", bufs=2) as xa0,
            tc.tile_pool(name="xa1", bufs=2) as xa1,
            tc.tile_pool(name="stg", bufs=3) as stg,
            tc.tile_pool(name="psA", bufs=2, space="PSUM") as psA,
        ):
            for c in range(NCHUNK):
                for g in range(NGR):
                    n0 = g * GRP * P
                    xts = []
                    for k, pool_k in ((0, xa0), (1, xa1)):
                        t = pool_k.tile([P, GRP * P], bf16)
                        nc.sync.dma_start(
                            out=t[:],
                            in_=xt_d[
                                k * P : (k + 1) * P,
                                c * CHP + n0 : c * CHP + n0 + GRP * P,
                            ],
                        )
                        xts.append(t)
                    ps = psA.tile([P, GRP, 72], f32, space="PSUM")
                    for i in range(GRP):
                        for k in range(2):
                            nc.tensor.matmul(
                                out=ps[:, i, :],
                                lhsT=xts[k][:, i * P : (i + 1) * P],
                                rhs=Wp[k][:],
                                start=(k == 0),
                                stop=False,
                            )
                        nc.tensor.matmul(
                            out=ps[:, i, :],
                            lhsT=ones1[:],
                            rhs=b72[:],
                            start=False,
                            stop=True,
                        )
                    st = stg.tile([P, GRP, 72], bf16)
                    nc.vector.tensor_copy(out=st[:], in_=ps[:])
                    nc.sync.dma_start(
                        out=tbl[c][n0 : n0 + GRP * P, 0:72].rearrange(
                            "(g p) d -> p g d", p=P
                        ),
                        in_=st[:],
                    )

        # ---- edge phase ----
        seg_reg = nc.gpsimd.to_reg(SEG)
        DBG = KDBG
        with (
            tc.tile_pool(name="ix", bufs=2) as ixp,
            tc.tile_pool(name="sb", bufs=2) as sbp,
            tc.tile_pool(name="g", bufs=2) as gp,
            tc.tile_pool(name="rv", bufs=2) as rvp,
            tc.tile_pool(name="ex", bufs=2) as exp_,
            tc.tile_pool(name="z", bufs=4) as zp,
            tc.tile_pool(name="on", bufs=2) as onp,
            tc.tile_pool(name="psS", bufs=2, space="PSUM") as psS,
            tc.tile_pool(name="psU", bufs=2, space="PSUM") as psU,
        ):
            for q in range(NWP if DBG != 1 else 0):
                idx_sb = ixp.tile([P, NCHUNK * S16], i16)
                nc.sync.dma_start(
                    out=idx_sb[:],
                    in_=idx_d[:, q * NCHUNK * S16 : (q + 1) * NCHUNK * S16],
                )
                stbt = sbp.tile([P, 2, 2, T_w, P], fp8)
                nc.sync.dma_start(
                    out=stbt[:].rearrange("p a b t e -> p (a b t e)"),
                    in_=stb_d[
                        :, (2 * q) * 2 * T_w * P : (2 * q + 2) * 2 * T_w * P
                    ],
                )
                G = gp.tile([P, NCHUNK, 2, T_c, P], bf16)
                for c in range(NCHUNK):
                    nc.gpsimd.dma_gather(
                        G[:, c].rearrange("p a t e -> p (a t) e"),
                        tbl[c][:, :],
                        idx_sb[:, c * S16 : (c + 1) * S16],
                        SEG,
                        seg_reg,
                        P,
                        queue_num=c,
                        single_packet=False,
                    )
                for wi in range(2 if DBG == 0 or DBG >= 3 else 0):
                    w = 2 * q + wi
                    se_ps = psS.tile([P, T_w, L], f32, space="PSUM")
                    for t in range(T_w):
                        nc.tensor.matmul(
                            out=se_ps[:, t, :],
                            lhsT=stbt[:, wi, 1, t, :],
                            rhs=ssrc[:, w, :],
                            start=True,
                            stop=True,
                        )
                    # zt = -(se + sd + bias)  (score columns pre-negated)
                    zt = zp.tile([P, T_w, L], bf16)
                    nc.vector.tensor_tensor(
                        out=zt[:].rearrange("p (c t) l -> p c t l", c=NCHUNK),
                        in0=se_ps[:].rearrange("p (c t) l -> p c t l", c=NCHUNK),
                        in1=G[:, :, wi, :, D_EM : D_EM + L],
                        op=OP.add,
                    )
                    # sigmoid = 1 / (1 + exp(zt)) via Exp-only table
                    ue = zp.tile([P, T_w, L], bf16)
                    nc.scalar.activation(out=ue[:], in_=zt[:], func=AF.Exp)
                    u1 = zp.tile([P, T_w, L], bf16)
                    nc.vector.tensor_scalar(
                        out=u1[:], in0=ue[:], scalar1=1.0, scalar2=None, op0=OP.add
                    )
                    sig = zp.tile([P, T_w, L], bf16)
                    with nc.allow_low_precision(
                        reason="bf16 sigmoid; 2e-2 output tolerance"
                    ):
                        nc.vector.reciprocal(out=sig[:], in_=u1[:])
                    Rev = rvp.tile([P, T_w, 4 * D_EM + L], bf16)
                    nc.scalar.activation(
                        out=Rev[:, :, 4 * D_EM : 4 * D_EM + L],
                        in_=sig[:],
                        func=AF.Exp,
                    )
                    # ev_l * hem
                    if DBG != 0 and DBG < 4:
                        continue
                    if w % 3 != 0:
                        # Act expands ev -> stride-1 operand, DVE fast mult
                        evx = exp_.tile([P, T_w, L, D_EM], bf16)
                        nc.scalar.activation(
                            out=evx[:],
                            in_=sig[:].unsqueeze(3).to_broadcast(
                                [P, T_w, L, D_EM]
                            ),
                            func=AF.Exp,
                        )
                        for c in range(NCHUNK):
                            nc.vector.tensor_tensor(
                                out=Rev[:, c * T_c : (c + 1) * T_c, 0 : 4 * D_EM]
                                .rearrange("p t (l d) -> p t l d", l=L),
                                in0=G[:, c, wi, :, 0:D_EM]
                                .unsqueeze(2)
                                .to_broadcast([P, T_c, L, D_EM]),
                                in1=evx[:, c * T_c : (c + 1) * T_c],
                                op=OP.mult,
                            )
                    else:
                        for c in range(NCHUNK):
                            nc.vector.tensor_tensor(
                                out=Rev[:, c * T_c : (c + 1) * T_c, 0 : 4 * D_EM]
                                .rearrange("p t (l d) -> p t l d", l=L),
                                in0=G[:, c, wi, :, 0:D_EM]
                                .unsqueeze(2)
                                .to_broadcast([P, T_c, L, D_EM]),
                                in1=Rev[
                                    :, c * T_c : (c + 1) * T_c, 4 * D_EM :
                                ]
                                .unsqueeze(3)
                                .to_broadcast([P, T_c, L, D_EM]),
                                op=OP.mult,
                            )
                    if DBG != 0 and DBG < 5:
                        continue
                    U_ps = psU.tile([P, 4 * D_EM + L], f32, space="PSUM")
                    for t in range(T_w):
                        nc.tensor.matmul(
                            out=U_ps[:],
                            lhsT=stbt[:, wi, 0, t, :],
                            rhs=Rev[:, t, :],
                            start=(t == 0),
                            stop=(t == T_w - 1),
                        )
                    dn = onp.tile([P, L], f32)
                    nc.vector.tensor_scalar(
                        out=dn[:],
                        in0=U_ps[:, 4 * D_EM : 4 * D_EM + L],
                        scalar1=1e-30,
                        scalar2=None,
                        op0=OP.add,
                    )
                    dnr = onp.tile([P, L], f32)
                    nc.vector.reciprocal(out=dnr[:], in_=dn[:])
                    ot = onp.tile([P, 4 * D_EM], f32)
                    nc.vector.tensor_tensor(
                        out=ot[:].rearrange("p (l d) -> p l d", l=L),
                        in0=U_ps[:, 0 : 4 * D_EM].rearrange("p (l d) -> p l d", l=L),
                        in1=dnr[:].unsqueeze(2).to_broadcast([P, L, D_EM]),
                        op=OP.mult,
                    )
                    nc.sync.dma_start(
                        out=out_d[w * P : (w + 1) * P, :], in_=ot[:]
                    )

    # gpsimd ucode library loads for dma_gather + ISA byte codegen
    inst_type_to_lib_mask = {}
    for lib in all_libraries:
        for inst_type in lib.instructions:
            inst_type_to_lib_mask[inst_type] = inst_type_to_lib_mask.get(
                inst_type, 0
            ) | (1 << lib.index)
    bass._bass_rust.insert_library_loads(
        nc, inst_type_to_lib_mask, len(all_libraries), standard.index
    )
    mybir.codegen_inst_isa_subclasses(nc)
    _split_multi_waits(nc)
    return nc


# ----------------------------------------------------------------------------
# public entry point
# ----------------------------------------------------------------------------

_NC_CACHE = {}


def _get_nc(N, NW, T_w, TC=None):
    key = (N, NW, T_w)
    if key not in _NC_CACHE:
        _NC_CACHE[key] = _build_nc(N, NW, T_w)
    return _NC_CACHE[key]


def _make_in_maps(x, W_lin, b_lin, att_w, att_b, emb_w, emb_b, per_core, n_cores):
    x = np.asarray(x, np.float32)
    N = x.shape[0]
    NV = N // n_cores
    NW = (NV + P - 1) // P
    CHP = ((N // NCHUNK) + P - 1) // P * P
    NP = NCHUNK * CHP
    chrows = N // NCHUNK

    W_lin = np.asarray(W_lin, np.float32)
    att_w = np.asarray(att_w, np.float32)
    a_src = att_w[:, :IN_F]
    a_dst = att_w[:, IN_F:]
    wp = np.concatenate(
        [
            np.asarray(emb_w, np.float32),
            -(W_lin @ a_dst.T),
            -(W_lin @ a_src.T),
        ],
        axis=1,
    )
    bl = np.asarray(b_lin, np.float32)
    b72 = np.concatenate(
        [
            np.asarray(emb_b, np.float32),
            -(bl @ a_dst.T + np.asarray(att_b, np.float32)),
            -(bl @ a_src.T),
        ]
    ).reshape(1, 72)

    xT = x.T.astype(ml_dtypes.bfloat16)
    xtp = np.zeros((IN_F, NP), ml_dtypes.bfloat16)
    for c in range(NCHUNK):
        xtp[:, c * CHP : c * CHP + chrows] = xT[:, c * chrows : (c + 1) * chrows]
    shared = dict(
        xt=xtp,
        wp=wp.astype(ml_dtypes.bfloat16),
        b72=b72.astype(ml_dtypes.bfloat16),
    )
    in_maps = []
    for c in range(n_cores):
        pc = per_core[c]
        xtu = np.zeros((IN_F, NW * P), ml_dtypes.bfloat16)
        xtu[:, : pc["K"]] = x[c * NV + pc["u"]].T.astype(ml_dtypes.bfloat16)
        m = dict(shared)
        m["xtu"] = xtu
        m["idx16"] = pc["idx16"]
        m["stb"] = pc["stb"]
        in_maps.append(m)
    return in_maps


def kernel(x, src, dst, W_lin, b_lin, att_w, att_b, emb_w, emb_b):
    from concourse.bass_utils import run_bass_kernel_spmd

    x = np.asarray(x)
    N = x.shape[0]
    cfg, per_core = _host_prep(src, dst, N, CORES)
    nc = _get_nc(N, cfg["NW"], cfg["T_w"])
    in_maps = _make_in_maps(
        x, W_lin, b_lin, att_w, att_b, emb_w, emb_b, per_core, CORES
    )
    res = run_bass_kernel_spmd(nc, in_maps, list(range(CORES)))
    out = np.zeros((N, 4 * D_EM), np.float32)
    NV = cfg["NV"]
    for c in range(CORES):
        K = per_core[c]["K"]
        u = per_core[c]["u"]
        out[c * NV + u] = res.results[c]["out"][:K]
    return out


# revision 19
# speedup vs baseline: 2.2030x; 1.0242x over previous
"""Bass/Trainium2 kernel for nn_DisentangleLayer (FactorGCN-style GNN layer).

Math (per reference):
  h    = x @ W_lin + b_lin                    [N, 256]
  h_em = x @ emb_w + emb_b                    [N, 64]
  s_src = h @ a_src.T ; s_dst = h @ a_dst.T   [N, 4]    (att_w = [a_src | a_dst])
  e    = sigmoid(s_src[src] + s_dst[dst] + att_b)       [E, 4]
  ev   = exp(e)              (the reference's max subtraction cancels in the
                              normalized ratio; sigmoid output is bounded)
  denom = segsum_src(ev)                       [N, 4]
  out[n, 64l:64l+64] = segsum_src(ev_l * h_em[dst]) / denom[n, l]

Weight folding (host, fp32).  Score columns are NEGATED so the device only
ever needs Exp (one activation table, no sigmoid<->exp table reloads):
  Wp  = [emb_w | -W_lin@a_dst.T | -W_lin@a_src.T]       [256, 72]
  b72 = [emb_b | -(b@a_dst.T+att_b) | -b@a_src.T]       [1, 72]
  tbl[n] = x[n] @ Wp + b72 = [hem(64) | -sd(4) | -ssrc(4)]
  u = exp(-(se+sd+b)) ; sigmoid = 1/(1+u) ; ev = exp(sigmoid)

Device strategy (per core; SPMD x8, core owns src nodes [c*NV,(c+1)*NV)):
  * Phase A: tbl [N, 128] bf16 rows (cols 72:128 unwritten pad for the
    256B-aligned gather), stored as 4 dst-chunk DRAM tensors so edge-phase
    gathers of chunk c can start as soon as chunk c is written.
    A separate compact pass in rank order (host passes x[u].T) produces
    per-window ssrc [128, 4] tiles directly into SBUF -- no gather.
  * Edge phase: edges sorted by src are mapped to dense ranks, grouped in
    128-rank windows; within a window slots are grouped by dst-chunk
    (25088 rows each -> int16 indices for dma_gather) and padded to T_c
    128-slot tiles per chunk.  Per window-pair, 4 batched dma_gather
    launches fetch tbl[dst] rows.  One-hot S (slot->rank) and its
    transpose ST stream from DRAM as fp8e4 and feed TensorE directly:
      se = ST.T @ ssrc ; ev = exp(sigmoid(se + sd)) ;
      U  = sum_t S_t.T @ [ev_l*hem | ev] ; out = U[:, :256] / denom.
    The ev_l*hem product: for 2 of 3 windows Act materializes an expanded
    ev (stride-1 last dim -> DVE 2x mode on the multiply); the rest run
    the broadcast multiply directly on DVE, balancing Act vs DVE.
"""

import math
import os
import numpy as np
import ml_dtypes
from contextlib import ExitStack

KDBG = int(os.environ.get("KDBG", "0"))  # 0=full, 1=phaseA only, 2=+gather,
                                         # 3=+se/exp, 4=+revmult, 5=+U/out

P = 128
CORES = 8
IN_F = 256
D_EM = 64
L = 4
NCHUNK = 4

_PATCHED = False


def _apply_tile_patch():
    """walrus in this env rejects >1 sem-wait on one instruction; split the
    TileContext exit-drain waits across single-wait nops."""
    global _PATCHED
    if _PATCHED:
        return
    _PATCHED = True
    import concourse.tile as tile_mod
    import concourse.mybir as mybir
    from concourse.vector_clock import ScopedClock

    def _drain_and_barrier(self, tick_clock, wait_clock):
        nop = self.nc.sync.nop()
        wait_clock.add_sem_waits(nop.ins, ScopedClock({None: tick_clock.global_clock}))
        si = nop.ins.sync_info
        waits = list(si.on_wait) if si is not None else []
        if len(waits) > 1:
            si.on_wait = waits[:1]
            nop.ins.sync_info = si
            for i in range(1, len(waits)):
                extra = self.nc.sync.nop()
                extra.ins.sync_info = mybir.SyncInfo(
                    on_wait=waits[i : i + 1], on_update=[]
                )
        self.nc.sync.drain()
        self.nc.all_engine_barrier()
        assert self.sems is not None
        popped = self.nc._tile_sem_poison_stack.pop()
        assert popped is self._sem_poison
        self.nc.clear_and_free_semaphores(list(self.sems.allocated().values()))
        self.nc.all_engine_barrier()

    tile_mod.TileContext._drain_and_barrier = _drain_and_barrier


def _split_multi_waits(nc):
    """This env's walrus accepts at most ONE sync-wait command per
    instruction.  Move extra waits onto single-wait nops inserted just
    before the instruction on the same engine (same sequencer => identical
    semantics)."""
    import concourse.mybir as mybir

    cnt = 0
    for f in nc.m.functions:
        for blk in f.blocks:
            insts = blk.instructions
            out = []
            changed = False
            for ins in insts:
                si = ins.sync_info
                waits = list(si.on_wait) if si is not None else []
                if len(waits) > 1:
                    changed = True
                    for w in waits[:-1]:
                        cnt += 1
                        nop = mybir.InstNoOp(
                            name=f"wsplit_{cnt}", ins=[], outs=[]
                        )
                        nop.engine = ins.engine
                        nop.sync_info = mybir.SyncInfo(on_wait=[w], on_update=[])
                        out.append(nop)
                    si.on_wait = waits[-1:]
                    ins.sync_info = si
                out.append(ins)
            if changed:
                blk.instructions = out
    return cnt


# ----------------------------------------------------------------------------
# host-side sharding / stream building
# ----------------------------------------------------------------------------

def _wrap_idx16(vals):
    """[n] int array -> dma_gather idx layout [128, n//16] int16:
    value j at [j%16, j//16], replicated across the 8 16-partition groups."""
    n = len(vals)
    m = np.zeros((16, n // 16), np.int16)
    m[np.arange(n) % 16, np.arange(n) // 16] = vals.astype(np.int16)
    return np.tile(m, (8, 1))


def _host_prep(src, dst, n_nodes, n_cores):
    NV = n_nodes // n_cores                       # 12500
    NW = (NV + P - 1) // P                        # 98
    src = np.asarray(src)
    dst = np.asarray(dst)
    chrows = n_nodes // NCHUNK                    # 25000

    cores = []
    for c in range(n_cores):
        lo = c * NV
        sel = (src >= lo) & (src < lo + NV)
        es = src[sel].astype(np.int64) - lo
        ed = dst[sel].astype(np.int64)
        order = np.argsort(es, kind="stable")
        es = es[order]
        ed = ed[order]
        u, counts = np.unique(es, return_counts=True)
        K = len(u)
        ranks = np.repeat(np.arange(K, dtype=np.int64), counts)
        w = ranks // P
        rw = ranks % P
        ch = ed // chrows
        seg = w * NCHUNK + ch
        o2 = np.lexsort((ranks, seg))  # group by (window, chunk)
        ed, w, rw, ch, seg = ed[o2], w[o2], rw[o2], ch[o2], seg[o2]
        cnt_seg = np.bincount(seg, minlength=NW * NCHUNK)
        cores.append((ed, u, K, w, rw, ch, seg, cnt_seg))

    T_c = 1
    for (_, _, _, _, _, _, _, cnt_seg) in cores:
        T_c = max(T_c, int(math.ceil(cnt_seg.max() / P)))
    T_w = NCHUNK * T_c
    NWP = NW // 2
    SEG = 2 * T_c * P  # indices per (pair, chunk) launch

    per_core = []
    for c in range(n_cores):
        ed, u, K, w, rw, ch, seg, cnt_seg = cores[c]
        offs_base = np.concatenate([[0], np.cumsum(cnt_seg)])
        pos = np.arange(len(ed)) - offs_base[seg]   # position within segment
        tc = pos // P                                # tile within chunk
        pp = pos % P                                 # slot partition

        # gather index stream, padded with 0 (gathers a real row; its S
        # column is zero so it contributes nothing)
        idxf = np.zeros((NW, NCHUNK, T_c * P), np.int16)
        idxf[w, ch, pos] = (ed - ch * chrows).astype(np.int16)
        idx16 = np.zeros((P, NWP * NCHUNK * (SEG // 16)), np.int16)
        s16 = SEG // 16
        for q in range(NWP):
            for cc in range(NCHUNK):
                flat = np.concatenate([idxf[2 * q, cc], idxf[2 * q + 1, cc]])
                col0 = (q * NCHUNK + cc) * s16
                idx16[:, col0 : col0 + s16] = _wrap_idx16(flat)

        # one-hot S and ST per window, fp8e4, layout [P, NW, 2, T_w, P]
        stb = np.zeros((P, NW, 2, T_w, P), ml_dtypes.float8_e4m3)
        tg = ch * T_c + tc
        stb[pp, w, 0, tg, rw] = 1.0
        stb[rw, w, 1, tg, pp] = 1.0
        stb = stb.reshape(P, NW * 2 * T_w * P)

        per_core.append(dict(idx16=idx16, stb=stb, u=u, K=K))

    cfg = dict(NV=NV, NW=NW, T_w=T_w)
    return cfg, per_core


# ----------------------------------------------------------------------------
# device program
# ----------------------------------------------------------------------------

def _build_nc(N, NW, T_w):
    _apply_tile_patch()
    import concourse.bass as bass
    import concourse.mybir as mybir
    import concourse.tile as tile
    from concourse.library_config import all_libraries, standard

    f32 = mybir.dt.float32
    bf16 = mybir.dt.bfloat16
    fp8 = mybir.dt.float8e4
    i16 = mybir.dt.int16
    AF = mybir.ActivationFunctionType
    OP = mybir.AluOpType

    T_c = T_w // NCHUNK
    NWP = NW // 2
    SEG = 2 * T_c * P
    S16 = SEG // 16
    CHP = ((N // NCHUNK) + P - 1) // P * P   # 25088 padded chunk rows
    NP = NCHUNK * CHP
    CHT = CHP // P                           # tiles per chunk (196)
    GRP = next(g for g in (6, 5, 4, 7, 3, 2, 1) if CHT % g == 0)
    NGR = CHT // GRP                         # groups per chunk (28)
    assert NW % 2 == 0

    nc = bass.Bass(num_swdge_queues=4)
    xt_d = nc.declare_dram_parameter("xt", [IN_F, NP], bf16, isOutput=False)
    xtu_d = nc.declare_dram_parameter("xtu", [IN_F, NW * P], bf16, isOutput=False)
    wp_d = nc.declare_dram_parameter("wp", [IN_F, 72], bf16, isOutput=False)
    b72_d = nc.declare_dram_parameter("b72", [1, 72], bf16, isOutput=False)
    idx_d = nc.declare_dram_parameter(
        "idx16", [P, NWP * NCHUNK * S16], i16, isOutput=False
    )
    stb_d = nc.declare_dram_parameter(
        "stb", [P, NW * 2 * T_w * P], fp8, isOutput=False
    )
    out_d = nc.declare_dram_parameter("out", [NW * P, 4 * D_EM], f32, isOutput=True)

    tbl = [nc.dram_tensor(f"tbl{c}", [CHP, P], bf16) for c in range(NCHUNK)]

    with ExitStack() as ctx:
        tc = ctx.enter_context(tile.TileContext(nc))
        const = ctx.enter_context(tc.tile_pool(name="const", bufs=1))

        Wp = [const.tile([P, 72], bf16, name=f"Wp{i}") for i in range(2)]
        for i in range(2):
            nc.sync.dma_start(out=Wp[i][:], in_=wp_d[i * P : (i + 1) * P, :])
        b72 = const.tile([1, 72], bf16)
        nc.sync.dma_start(out=b72[:], in_=b72_d[:])
        ones1 = const.tile([1, P], bf16)
        nc.gpsimd.memset(ones1[:], 1.0)
        ssrc = const.tile([P, NW, L], bf16)

        # ---- phase A-u: per-window (negated) ssrc tiles, rank-ordered ----
        with (
            tc.tile_pool(name="xu0", bufs=2) as xu0,
            tc.tile_pool(name="xu1", bufs=2) as xu1,
            tc.tile_pool(name="psu", bufs=2, space="PSUM") as psu,
        ):
            UB = 8  # windows per load group
            for g in range((NW + UB - 1) // UB):
                w0 = g * UB
                nw = min(UB, NW - w0)
                xu_sb = xu0.tile([P, 2, UB * P], bf16)
                nc.sync.dma_start(
                    out=xu_sb[:, :, : nw * P],
                    in_=xtu_d[:, w0 * P : (w0 + nw) * P].rearrange(
                        "(a k) n -> k a n", a=2
                    ),
                )
                for wi in range(nw):
                    w = w0 + wi
                    ps = psu.tile([P, 72], f32, space="PSUM")
                    for k in range(2):
                        nc.tensor.matmul(
                            out=ps[:],
                            lhsT=xu_sb[:, k, wi * P : (wi + 1) * P],
                            rhs=Wp[k][:],
                            start=(k == 0),
                            stop=False,
                        )
                    nc.tensor.matmul(
                        out=ps[:], lhsT=ones1[:], rhs=b72[:], start=False, stop=True
                    )
                    nc.vector.tensor_copy(
                        out=ssrc[:, w, :], in_=ps[:, D_EM + L : D_EM + 2 * L]
                    )

        # ---- phase A: tbl rows (chunk-major so gathers can pipeline) ----
        with (
            tc.tile_pool(name="xa0", bufs=2) as xa0,
            tc.tile_pool(name="xa1", bufs=2) as xa1,
            tc.tile_pool(name="stg", bufs=3) as stg,
            tc.tile_pool(name="psA", bufs=2, space="PSUM") as psA,
        ):
            for c in range(NCHUNK):
                for g2 in range((NGR + 1) // 2):
                    ng = 2 if 2 * g2 + 1 < NGR else 1
                    n0 = g2 * 2 * GRP * P
                    xt_sb = xa0.tile([P, 2, 2 * GRP * P], bf16)
                    nc.sync.dma_start(
                        out=xt_sb[:, :, : ng * GRP * P],
                        in_=xt_d[:, c * CHP + n0 : c * CHP + n0 + ng * GRP * P]
                        .rearrange("(a k) n -> k a n", a=2),
                    )
                    st = stg.tile([P, 2 * GRP, 72], bf16)
                    for h in range(ng):
                        ps = psA.tile([P, GRP, 72], f32, space="PSUM")
                        for i in range(GRP):
                            for k in range(2):
                                nc.tensor.matmul(
                                    out=ps[:, i, :],
                                    lhsT=xt_sb[
                                        :, k, (h * GRP + i) * P : (h * GRP + i + 1) * P
                                    ],
                                    rhs=Wp[k][:],
                                    start=(k == 0),
                                    stop=False,
                                )
                            nc.tensor.matmul(
                                out=ps[:, i, :],
                                lhsT=ones1[:],
                                rhs=b72[:],
                                start=False,
                                stop=True,
                            )
                        nc.vector.tensor_copy(
                            out=st[:, h * GRP : (h + 1) * GRP, :], in_=ps[:]
                        )
                    nc.sync.dma_start(
                        out=tbl[c][n0 : n0 + ng * GRP * P, 0:72].rearrange(
                            "(g p) d -> p g d", p=P
                        ),
                        in_=st[:, : ng * GRP, :],
                    )

        # ---- edge phase ----
        seg_reg = nc.gpsimd.to_reg(SEG)
        DBG = KDBG
        with (
            tc.tile_pool(name="ix", bufs=2) as ixp,
            tc.tile_pool(name="sb", bufs=2) as sbp,
            tc.tile_pool(name="g", bufs=2) as gp,
            tc.tile_pool(name="rv", bufs=2) as rvp,
            tc.tile_pool(name="ex", bufs=2) as exp_,
            tc.tile_pool(name="z", bufs=4) as zp,
            tc.tile_pool(name="on", bufs=2) as onp,
            tc.tile_pool(name="psS", bufs=2, space="PSUM") as psS,
            tc.tile_pool(name="psU", bufs=2, space="PSUM") as psU,
        ):
            for q in range(NWP if DBG != 1 else 0):
                idx_sb = ixp.tile([P, NCHUNK * S16], i16)
                nc.sync.dma_start(
                    out=idx_sb[:],
                    in_=idx_d[:, q * NCHUNK * S16 : (q + 1) * NCHUNK * S16],
                )
                stbt = sbp.tile([P, 2, 2, T_w, P], fp8)
                nc.sync.dma_start(
                    out=stbt[:].rearrange("p a b t e -> p (a b t e)"),
                    in_=stb_d[
                        :, (2 * q) * 2 * T_w * P : (2 * q + 2) * 2 * T_w * P
                    ],
                )
                G = gp.tile([P, NCHUNK, 2, T_c, P], bf16)
                for c in range(NCHUNK):
                    nc.gpsimd.dma_gather(
                        G[:, c].rearrange("p a t e -> p (a t) e"),
                        tbl[c][:, :],
                        idx_sb[:, c * S16 : (c + 1) * S16],
                        SEG,
                        seg_reg,
                        P,
                        queue_num=c,
                        single_packet=False,
                    )
                for wi in range(2 if DBG == 0 or DBG >= 3 else 0):
                    w = 2 * q + wi
                    se_ps = psS.tile([P, T_w, L], f32, space="PSUM")
                    for t in range(T_w):
                        nc.tensor.matmul(
                            out=se_ps[:, t, :],
                            lhsT=stbt[:, wi, 1, t, :],
                            rhs=ssrc[:, w, :],
                            start=True,
                            stop=True,
                        )
                    # zt = -(se + sd + bias)  (score columns pre-negated)
                    zt = zp.tile([P, T_w, L], bf16)
                    nc.vector.tensor_tensor(
                        out=zt[:].rearrange("p (c t) l -> p c t l", c=NCHUNK),
                        in0=se_ps[:].rearrange("p (c t) l -> p c t l", c=NCHUNK),
                        in1=G[:, :, wi, :, D_EM : D_EM + L],
                        op=OP.add,
                    )
                    # sigmoid = 1 / (1 + exp(zt)) via Exp-only table
                    ue = zp.tile([P, T_w, L], bf16)
                    nc.scalar.activation(out=ue[:], in_=zt[:], func=AF.Exp)
                    u1 = zp.tile([P, T_w, L], bf16)
                    nc.vector.tensor_scalar(
                        out=u1[:], in0=ue[:], scalar1=1.0, scalar2=None, op0=OP.add
                    )
                    sig = zp.tile([P, T_w, L], bf16)
                    with nc.allow_low_precision(
                        reason="bf16 sigmoid; 2e-2 output tolerance"
                    ):
                        nc.vector.reciprocal(out=sig[:], in_=u1[:])
                    Rev = rvp.tile([P, T_w, 4 * D_EM + L], bf16)
                    nc.scalar.activation(
                        out=Rev[:, :, 4 * D_EM : 4 * D_EM + L],
                        in_=sig[:],
                        func=AF.Exp,
                    )
                    # ev_l * hem
                    if DBG != 0 and DBG < 4:
                        continue
                    if w % 7 != 0:
                        # Act expands ev -> stride-1 operand, DVE fast mult
                        evx = exp_.tile([P, T_w, L, D_EM], bf16)
                        nc.scalar.activation(
                            out=evx[:],
                            in_=sig[:].unsqueeze(3).to_broadcast(
                                [P, T_w, L, D_EM]
                            ),
                            func=AF.Exp,
                        )
                        for c in range(NCHUNK):
                            nc.vector.tensor_tensor(
                                out=Rev[:, c * T_c : (c + 1) * T_c, 0 : 4 * D_EM]
                                .rearrange("p t (l d) -> p t l d", l=L),
                                in0=G[:, c, wi, :, 0:D_EM]
                                .unsqueeze(2)
                                .to_broadcast([P, T_c, L, D_EM]),
                                in1=evx[:, c * T_c : (c + 1) * T_c],
                                op=OP.mult,
                            )
                    else:
                        for c in range(NCHUNK):
                            nc.vector.tensor_tensor(
                                out=Rev[:, c * T_c : (c + 1) * T_c, 0 : 4 * D_EM]
                                .rearrange("p t (l d) -> p t l d", l=L),
                                in0=G[:, c, wi, :, 0:D_EM]
                                .unsqueeze(2)
                                .to_broadcast([P, T_c, L, D_EM]),
                                in1=Rev[
                                    :, c * T_c : (c + 1) * T_c, 4 * D_EM :
                                ]
                                .unsqueeze(3)
                                .to_broadcast([P, T_c, L, D_EM]),
                                op=OP.mult,
                            )
                    if DBG != 0 and DBG < 5:
                        continue
                    U_ps = psU.tile([P, 4 * D_EM + L], f32, space="PSUM")
                    for t in range(T_w):
                        nc.tensor.matmul(
                            out=U_ps[:],
                            lhsT=stbt[:, wi, 0, t, :],
                            rhs=Rev[:, t, :],
                            start=(t == 0),
                            stop=(t == T_w - 1),
                        )
                    dn = onp.tile([P, L], f32)
                    nc.vector.tensor_scalar(
                        out=dn[:],
                        in0=U_ps[:, 4 * D_EM : 4 * D_EM + L],
                        scalar1=1e-30,
                        scalar2=None,
                        op0=OP.add,
                    )
                    dnr = onp.tile([P, L], f32)
                    nc.vector.reciprocal(out=dnr[:], in_=dn[:])
                    ot = onp.tile([P, 4 * D_EM], f32)
                    nc.vector.tensor_tensor(
                        out=ot[:].rearrange("p (l d) -> p l d", l=L),
                        in0=U_ps[:, 0 : 4 * D_EM].rearrange("p (l d) -> p l d", l=L),
                        in1=dnr[:].unsqueeze(2).to_broadcast([P, L, D_EM]),
                        op=OP.mult,
                    )
                    nc.sync.dma_start(
                        out=out_d[w * P : (w + 1) * P, :], in_=ot[:]
                    )

    # gpsimd ucode library loads for dma_gather + ISA byte codegen
    inst_type_to_lib_mask = {}
    for lib in all_libraries:
        for inst_type in lib.instructions:
            inst_type_to_lib_mask[inst_type] = inst_type_to_lib_mask.get(
                inst_type, 0
            ) | (1 << lib.index)
    bass._bass_rust.insert_library_loads(
        nc, inst_type_to_lib_mask, len(all_libraries), standard.index
    )
    mybir.codegen_inst_isa_subclasses(nc)
    _split_multi_waits(nc)
    return nc


# ----------------------------------------------------------------------------
# public entry point
# ----------------------------------------------------------------------------

_NC_CACHE = {}


def _get_nc(N, NW, T_w, TC=None):
    key = (N, NW, T_w)
    if key not in _NC_CACHE:
        _NC_CACHE[key] = _build_nc(N, NW, T_w)
    return _NC_CACHE[key]


def _make_in_maps(x, W_lin, b_lin, att_w, att_b, emb_w, emb_b, per_core, n_cores):
    x = np.asarray(x, np.float32)
    N = x.shape[0]
    NV = N // n_cores
    NW = (NV + P - 1) // P
    CHP = ((N // NCHUNK) + P - 1) // P * P
    NP = NCHUNK * CHP
    chrows = N // NCHUNK

    W_lin = np.asarray(W_lin, np.float32)
    att_w = np.asarray(att_w, np.float32)
    a_src = att_w[:, :IN_F]
    a_dst = att_w[:, IN_F:]
    wp = np.concatenate(
        [
            np.asarray(emb_w, np.float32),
            -(W_lin @ a_dst.T),
            -(W_lin @ a_src.T),
        ],
        axis=1,
    )
    bl = np.asarray(b_lin, np.float32)
    b72 = np.concatenate(
        [
            np.asarray(emb_b, np.float32),
            -(bl @ a_dst.T + np.asarray(att_b, np.float32)),
            -(bl @ a_src.T),
        ]
    ).reshape(1, 72)

    xT = x.T.astype(ml_dtypes.bfloat16)
    xtp = np.zeros((IN_F, NP), ml_dtypes.bfloat16)
    for c in range(NCHUNK):
        xtp[:, c * CHP : c * CHP + chrows] = xT[:, c * chrows : (c + 1) * chrows]
    shared = dict(
        xt=xtp,
        wp=wp.astype(ml_dtypes.bfloat16),
        b72=b72.astype(ml_dtypes.bfloat16),
    )
    in_maps = []
    for c in range(n_cores):
        pc = per_core[c]
        xtu = np.zeros((IN_F, NW * P), ml_dtypes.bfloat16)
        xtu[:, : pc["K"]] = x[c * NV + pc["u"]].T.astype(ml_dtypes.bfloat16)
        m = dict(shared)
        m["xtu"] = xtu
        m["idx16"] = pc["idx16"]
        m["stb"] = pc["stb"]
        in_maps.append(m)
    return in_maps


def kernel(x, src, dst, W_lin, b_lin, att_w, att_b, emb_w, emb_b):
    from concourse.bass_utils import run_bass_kernel_spmd

    x = np.asarray(x)
    N = x.shape[0]
    cfg, per_core = _host_prep(src, dst, N, CORES)
    nc = _get_nc(N, cfg["NW"], cfg["T_w"])
    in_maps = _make_in_maps(
        x, W_lin, b_lin, att_w, att_b, emb_w, emb_b, per_core, CORES
    )
    res = run_bass_kernel_spmd(nc, in_maps, list(range(CORES)))
    out = np.zeros((N, 4 * D_EM), np.float32)
    NV = cfg["NV"]
    for c in range(CORES):
        K = per_core[c]["K"]
        u = per_core[c]["u"]
        out[c * NV + u] = res.results[c]["out"][:K]
    return out
